# revision 34
# baseline (speedup 1.0000x reference)
"""MoE (top-2 routed GluMLP) Trainium2 kernel, expert+DFF-parallel over 8 cores.

Contract: kernel(**inputs) takes the FULL unsharded inputs
  x  [2, 2048, 1024] f32
  Wr [8, 1024] f32           router
  Wg [8, 4096, 1024] f32     gate proj per expert
  Wu [8, 4096, 1024] f32     up proj per expert
  Wd [8, 1024, 4096] f32     down proj per expert
and returns the FULL output [2, 2048, 1024] f32.

Strategy:
  - Routing (softmax + top-2 + renormalize) on host with jax on CPU using the
    exact ops of the reference, so expert selection and combine weights match
    the reference bit-for-bit.
  - Load balancing exploits that the GluMLP is additive along DFF: every
    expert is split into two 16-row-of-128 (DFF/2) halves. The 4 heaviest
    experts' halves fill segment 1 on the 8 cores (capacity S1 = max load);
    the 4 lightest fill segment 2 (capacity S2 = 5th-largest load). Per-core
    work is 16*(S1+S2) fm-tokens instead of 32*maxload — ~3% less.
  - Each (core, segment) runs a weighted GluMLP half over its tokens:
        h = relu(x @ WgT_half) * (x @ WuT_half);  out_part = Wd_half @ h
    with fp16 operands (same 10-bit mantissa as TF32) and fp32 PSUM
    accumulation. Phase C keeps TOKENS as the moving operand (no 128-token
    tile quantization) with the Wd halves fully resident in SBUF.
  - The renormalized router weight is applied on the HOST during the
    scatter-add (it commutes through the down-projection), which also sums
    the two DFF-half partials per (token, expert).

Env: MOE_MM_DT selects matmul operand dtype (f16 default, f32r, f32).
"""

import math
import os
from contextlib import ExitStack

import numpy as np

import concourse.bass as bass
import concourse.tile as tile
from concourse import bacc, mybir
from concourse.bass_utils import run_bass_kernel_spmd

B, L, D, E, TOPK, DFF = 2, 2048, 1024, 8, 2, 4096
T = B * L
NCORES = 8
P = 128
NB = 512          # matmul moving-operand block (one PSUM bank of fp32)
DC = D // P       # 8 contraction chunks over D (also 8 output d-tiles)
FM = DFF // P     # 32 f-tiles over DFF per expert
FMH = FM // 2     # 16 f-tiles per expert half
NSEG = 2

F32 = mybir.dt.float32
F32R = mybir.dt.float32r
F16 = mybir.dt.float16

# Set to True (e.g. from test.py) to run with NTFF tracing and print HW time.
PROFILE = False
TRACE_CORES = None  # e.g. list(range(8)) to profile every core
LAST_EXEC_NS = None
MM_DT = {"f32": F32, "f32r": F32R, "f16": F16}[os.environ.get("MOE_MM_DT", "f16")]


def _nblocks(tch):
    """Moving-dim blocks <=512. Every block must stream >= ~240 cols so the
    97ns (233-cycle) LDWEIGHTS stays hidden behind the col+6-cycle stream.
    The LAST block is made the smallest allowed: the final (block, d-tile)
    accumulation chain + drain of phase C is a pure serial tail."""
    k = max(1, math.ceil(tch / NB))
    if k == 1:
        return [(0, tch)]
    last = max(240, tch - NB * (k - 1))
    rest = tch - last
    base, rem = divmod(rest, k - 1)
    out, t = [], 0
    for i in range(k - 1):
        nb = base + (1 if i < rem else 0)
        out.append((t, nb))
        t += nb
    out.append((t, last))
    return out


def _build_nc(S1: int, S2: int):
    """Build the single-core Bass program (SPMD: all cores run the same NEFF).

    Segment s processes S_s tokens against one 16-f-tile expert half whose
    weights are input slice s."""
    nc = bacc.Bacc(
        "TRN2",
        target_bir_lowering=False,
        debug=False,
        enable_asserts=False,
        num_devices=NCORES,
    )
    mdt = MM_DT
    S = S1 + S2
    x_d = nc.dram_tensor("x", [P, DC, S], mdt, kind="ExternalInput").ap()
    wg_d = nc.dram_tensor("wg", [NSEG, FMH, P, DC, P], mdt, kind="ExternalInput").ap()
    wu_d = nc.dram_tensor("wu", [NSEG, FMH, P, DC, P], mdt, kind="ExternalInput").ap()
    wd_d = nc.dram_tensor("wd", [NSEG, FMH, P, D], mdt, kind="ExternalInput").ap()
    # out is [d-tile, d-inner, t] (transposed); host transposes, applies the
    # router weight, and sums the two DFF-half partials in the scatter-add.
    out_d = nc.dram_tensor("out", [DC, P, S], F32, kind="ExternalOutput").ap()

    with tile.TileContext(nc) as tc:
        with ExitStack() as ctx:
            _moe_body(ctx, tc, x_d, wg_d, wu_d, wd_d, out_d, S1, S2)
    nc.compile()
    return nc


def _moe_body(ctx, tc, x_d, wg_d, wu_d, wd_d, out_d, S1, S2):
    nc = tc.nc
    mdt = MM_DT
    S = S1 + S2
    segs = [(0, 0, S1), (1, S1, S2)]  # (seg idx, token offset, token count)

    xpool = ctx.enter_context(tc.tile_pool(name="xpool", bufs=1))
    hpool = ctx.enter_context(tc.tile_pool(name="hpool", bufs=1))
    # bufs=4: the in-loop wg/wu load for fm+4 can dispatch as soon as fm's
    # buffer frees — shallower prefetch slips behind the startup queue backlog
    wgupool = ctx.enter_context(tc.tile_pool(name="wgupool", bufs=4))
    # all 16 Wd tiles (2 segs x 8 d-tiles, 4KiB/partition each) stay resident:
    # Wd streams exactly once, behind phase B, and phase C never waits on DMA
    wdpool = ctx.enter_context(tc.tile_pool(name="wdpool", bufs=NSEG * DC))
    gpool = ctx.enter_context(tc.tile_pool(name="gpool", bufs=3))
    opool = ctx.enter_context(tc.tile_pool(name="opool", bufs=6))
    # One shared PSUM pool: phase B (ps_g/ps_u) and phase C (d-tiles) don't
    # overlap in time, so both get all 8 banks.
    psP = ctx.enter_context(tc.tile_pool(name="psP", bufs=8, space="PSUM"))

    # ---- startup DMA schedule -------------------------------------------
    # Bandwidth bound: ~3.2MB (segment-1 x + first two wg/wu pairs) must land
    # in the first ~13us. Full-tile transfers only — partition-split halves
    # and short rows both measurably reduce per-queue throughput. wg0 and
    # x dc=0 head different queues; wu0/wu1 ride later/slower slots.
    # All 8 x stripes ride the two fast HWDGE queues (slow-SWDGE stripes were
    # the measured dc=1/dc=4 chain stalls); gpsimd carries only the weights
    # that aren't needed until ~2.4us (wu0), ~7us (wg1) and ~15us (wu1) in.
    wg_p0 = wgupool.tile([P, DC, P], mdt, tag="wg", name="wg_pre0")
    nc.sync.dma_start(out=wg_p0, in_=wg_d[0, 0])
    x_sb = xpool.tile([P, DC, S], mdt, tag="x")
    nc.scalar.dma_start(out=x_sb[:, 0, :S1], in_=x_d[:, 0, :S1])
    wu_p0 = wgupool.tile([P, DC, P], mdt, tag="wu", name="wu_pre0")
    nc.gpsimd.dma_start(out=wu_p0, in_=wu_d[0, 0])
    for dc in range(1, DC):
        eng = [None, nc.scalar, nc.sync, nc.scalar, nc.sync, nc.scalar,
               nc.sync, nc.scalar][dc]
        eng.dma_start(out=x_sb[:, dc, :S1], in_=x_d[:, dc, :S1])
    wg_p1 = wgupool.tile([P, DC, P], mdt, tag="wg", name="wg_pre1")
    nc.gpsimd.dma_start(out=wg_p1, in_=wg_d[0, 1])
    wu_p1 = wgupool.tile([P, DC, P], mdt, tag="wu", name="wu_pre1")
    nc.gpsimd.dma_start(out=wu_p1, in_=wu_d[0, 1])
    pre = [(wg_p0, wu_p0), (wg_p1, wu_p1)]

    h_all = hpool.tile([P, FMH, S], mdt, tag="h")
    wd_sbs = [[None] * DC for _ in range(NSEG)]

    # ---- Phase B: h[f, t] = relu(g) * u, f-major, per segment ------------
    for seg, off, sl in segs:
        for fm in range(FMH):
            if seg == 0 and fm < len(pre):
                wg_sb, wu_sb = pre[fm]
            else:
                wg_sb = wgupool.tile([P, DC, P], mdt, tag="wg")
                nc.sync.dma_start(out=wg_sb, in_=wg_d[seg, fm])
                wu_sb = wgupool.tile([P, DC, P], mdt, tag="wu")
                nc.scalar.dma_start(out=wu_sb, in_=wu_d[seg, fm])
            if seg == 0 and 3 <= fm < 3 + DC:
                # segment-2 x streams one stripe per fm iteration (needed
                # ~115us in; bursting them would starve the wg/wu prefetch)
                dc = fm - 3
                xeng = [nc.gpsimd, nc.sync, nc.scalar][dc % 3]
                xeng.dma_start(out=x_sb[:, dc, S1:], in_=x_d[:, dc, S1:])
            if seg == 1:
                # trickle the 16 resident Wd tiles behind segment 2's weight
                # stream; phase C runs segment 2 first, so its tiles load in
                # the first 8 slots
                ws, dt = divmod(fm, DC)
                ws = 1 - ws  # fm 0..7 -> seg2's wd, fm 8..15 -> seg1's
                wd_t = wdpool.tile([P, FMH, P], mdt, tag="wd", name=f"wd{ws}_{dt}")
                eng = [nc.sync, nc.scalar, nc.gpsimd][dt % 3]
                eng.dma_start(
                    out=wd_t,
                    in_=wd_d[ws, :, :, dt * P : dt * P + P].rearrange(
                        "f p d -> p f d"
                    ),
                )
                wd_sbs[ws][dt] = wd_t
            for nb0, nbl in _nblocks(sl):
                ts = slice(off + nb0, off + nb0 + nbl)
                ps_g = psP.tile([P, NB], F32, tag="ps")
                ps_u = psP.tile([P, NB], F32, tag="ps")
                for dc in range(DC):
                    nc.tensor.matmul(
                        ps_g[:, :nbl],
                        lhsT=wg_sb[:, dc, :],
                        rhs=x_sb[:, dc, ts],
                        start=(dc == 0),
                        stop=(dc == DC - 1),
                    )
                for dc in range(DC):
                    nc.tensor.matmul(
                        ps_u[:, :nbl],
                        lhsT=wu_sb[:, dc, :],
                        rhs=x_sb[:, dc, ts],
                        start=(dc == 0),
                        stop=(dc == DC - 1),
                    )
                g_sb = gpool.tile([P, NB], F32, tag="g")
                # relu on DVE (tensor_scalar_max), NOT the ACT engine: any
                # ACTIVATE instruction makes the scalar engine run a 1.28us
                # ACT_TABLE_LOAD in the preamble before its first DMA
                # dispatch, delaying the critical startup x stripes
                nc.vector.tensor_scalar_max(g_sb[:, :nbl], ps_g[:, :nbl], 0.0)
                nc.vector.tensor_mul(h_all[:, fm, ts], g_sb[:, :nbl], ps_u[:, :nbl])

    # ---- Phase C: out[d, t] = (Wd_half h)[d, t], tokens moving -----------
    # Segment 2 first: segment 1 ends with the small last block, so the
    # final serial chain + drain is as short as possible.
    st = 0
    for seg, off, sl in reversed(segs):
        blocks = _nblocks(sl)
        for bi, (nb0, nbl) in enumerate(blocks):
            # gpsimd's end-of-program SWDGE ring drain (~3.7us) starts only
            # after its last instruction: keep it off the final block's stores
            final_blk = seg == 0 and bi == len(blocks) - 1
            ts = slice(off + nb0, off + nb0 + nbl)
            for dt in range(DC):
                ps_o = psP.tile([P, NB], F32, tag="ps", name=f"ps_o{dt}")
                for fc in range(FMH):
                    nc.tensor.matmul(
                        ps_o[:, :nbl],
                        lhsT=wd_sbs[seg][dt][:, fc, :],
                        rhs=h_all[:, fc, ts],
                        start=(fc == 0),
                        stop=(fc == FMH - 1),
                    )
                o_sb = opool.tile([P, NB], F32, tag="o")
                # drains on DVE only (scalar.copy is ACTIVATE — see relu note;
                # DVE is otherwise idle in phase C). The final block drains in
                # partition halves so its first store (the serial program
                # tail) dispatches half a copy earlier.
                if final_blk:
                    hp = P // 2
                    nc.vector.tensor_scalar_mul(
                        o_sb[:hp, :nbl], ps_o[:hp, :nbl], 1.0
                    )
                    nc.vector.tensor_scalar_mul(
                        o_sb[hp:, :nbl], ps_o[hp:, :nbl], 1.0
                    )
                else:
                    nc.vector.tensor_scalar_mul(o_sb[:, :nbl], ps_o[:, :nbl], 1.0)
                # stores split into partition halves cycling the DMA queues
                # so the final block's drain isn't serialized
                engs = [nc.sync, nc.scalar] if final_blk else [
                    nc.gpsimd, nc.sync, nc.scalar]
                for hh in range(2):
                    rs = slice(hh * (P // 2), (hh + 1) * (P // 2))
                    engs[st % len(engs)].dma_start(
                        out=out_d[dt, rs, off + nb0 : off + nb0 + nbl],
                        in_=o_sb[rs, :nbl],
                    )
                    st += 1


_NC_CACHE: dict = {}


def _get_nc(S1: int, S2: int):
    if (S1, S2) not in _NC_CACHE:
        _NC_CACHE[(S1, S2)] = _build_nc(S1, S2)
    return _NC_CACHE[(S1, S2)]


def _round_tf32(a):
    """Round-to-nearest-even fp32 -> TF32 (10-bit mantissa), as np.float32."""
    u = a.astype(np.float32).view(np.uint32).astype(np.uint64)
    lsb = (u >> 13) & 1
    r = (u + 0x0FFF + lsb) & 0xFFFFE000
    return r.astype(np.uint32).view(np.float32)


def _mm_round(a):
    """Convert a host array to the dtype/value the device matmuls consume."""
    if MM_DT is F32R:
        return _round_tf32(a)
    if MM_DT is F16:
        return np.ascontiguousarray(a, dtype=np.float16)
    return np.ascontiguousarray(a, dtype=np.float32)


def _route_host(x, Wr):
    """Reference-identical routing on host (jax on CPU, same ops as reference).

    Returns (k_ids [T, K] int, k_w [T, K] f32).
    """
    import jax
    import jax.numpy as jnp

    cpu = jax.devices("cpu")[0]
    with jax.default_device(cpu):
        xt = jnp.asarray(x.reshape(T, D))
        logits = jnp.einsum("td,ed->te", xt, jnp.asarray(Wr))
        scores = jax.nn.softmax(logits, axis=-1)
        k_scores, k_ids = jax.lax.top_k(scores, TOPK)
        eps = jnp.finfo(x.dtype).eps
        k_w = k_scores / (k_scores.sum(axis=-1, keepdims=True) + eps)
        return np.asarray(k_ids), np.asarray(k_w)


def _prep_weights(Wg, Wu, Wd):
    """Per-expert weight tensors in device layouts (rounded to MM_DT)."""
    wg_r, wu_r, wd_r = [], [], []
    for e in range(len(Wg)):
        # Wg[e]: [DFF, D]; device wants [fm, p(d_inner), dc, f_inner]
        wgt = Wg[e].T.reshape(DC, P, FM, P).transpose(2, 1, 0, 3)
        wut = Wu[e].T.reshape(DC, P, FM, P).transpose(2, 1, 0, 3)
        # Wd[e]: [D, DFF]; device wants WdT = [fc, p(f_inner), d]
        wdt = Wd[e].T.reshape(FM, P, D)
        wg_r.append(_mm_round(np.ascontiguousarray(wgt, dtype=np.float32)))
        wu_r.append(_mm_round(np.ascontiguousarray(wut, dtype=np.float32)))
        wd_r.append(_mm_round(np.ascontiguousarray(wdt, dtype=np.float32)))
    return wg_r, wu_r, wd_r


def _xdev(xt_rows, cap):
    """Host [n, D] f32 -> device x layout [p(d_inner), dc, cap] in MM_DT."""
    xg = np.zeros((cap, D), dtype=np.float32)
    xg[: len(xt_rows)] = xt_rows
    xg_r = np.ascontiguousarray(
        xg.T.reshape(DC, P, cap).transpose(1, 0, 2), dtype=np.float32
    )
    return _mm_round(xg_r)


def kernel(x, Wr, Wg, Wu, Wd):
    global LAST_EXEC_NS
    x = np.asarray(x, dtype=np.float32)
    Wr = np.asarray(Wr, dtype=np.float32)
    Wg = np.asarray(Wg, dtype=np.float32)
    Wu = np.asarray(Wu, dtype=np.float32)
    Wd = np.asarray(Wd, dtype=np.float32)

    k_ids, k_w = _route_host(x, Wr)
    xt = x.reshape(T, D)

    # Gather per-expert token lists (each token appears once per selected expert).
    idx_lists, w_lists = [], []
    for e in range(E):
        tmask = k_ids == e                       # [T, K]
        tok = np.nonzero(tmask.any(axis=1))[0]   # unique tokens routed to e
        wvals = (k_w * tmask).sum(axis=1)[tok].astype(np.float32)
        idx_lists.append(tok)
        w_lists.append(wvals)

    loads = np.array([len(t) for t in idx_lists])
    order = np.argsort(-loads, kind="stable")
    # DFF-split balance: segment 1 carries the 4 heaviest experts (two cores
    # per expert, one 16-f-tile half each), segment 2 the 4 lightest.
    S1 = max(P, (int(loads[order[0]]) + 7) // 8 * 8)
    S2 = max(P, (int(loads[order[4]]) + 7) // 8 * 8)
    assert S1 + S2 <= 2304, f"capacity {S1}+{S2} exceeds SBUF budget"

    wg_r, wu_r, wd_r = _prep_weights(Wg, Wu, Wd)

    in_maps = []
    core_assign = []  # (expert_a, expert_b) per core
    for c in range(NCORES):
        a, ha = int(order[c // 2]), c % 2
        b, hb = int(order[4 + c // 2]), c % 2
        xa = _xdev(xt[idx_lists[a]], S1)
        xb = _xdev(xt[idx_lists[b]], S2)
        in_maps.append(
            {
                "x": np.concatenate([xa, xb], axis=2),
                "wg": np.ascontiguousarray(
                    np.stack([wg_r[a][FMH * ha : FMH * (ha + 1)],
                              wg_r[b][FMH * hb : FMH * (hb + 1)]])
                ),
                "wu": np.ascontiguousarray(
                    np.stack([wu_r[a][FMH * ha : FMH * (ha + 1)],
                              wu_r[b][FMH * hb : FMH * (hb + 1)]])
                ),
                "wd": np.ascontiguousarray(
                    np.stack([wd_r[a][FMH * ha : FMH * (ha + 1)],
                              wd_r[b][FMH * hb : FMH * (hb + 1)]])
                ),
            }
        )
        core_assign.append((a, b))

    nc = _get_nc(S1, S2)
    core_ids = list(range(NCORES))
    if PROFILE:
        res = _run_profiled(nc, in_maps, core_ids)
        LAST_EXEC_NS = res.exec_time_ns
        results = res.results
    else:
        results = run_bass_kernel_spmd(nc, in_maps, core_ids).results

    out = np.zeros((T, D), dtype=np.float32)
    for c in range(NCORES):
        a, b = core_assign[c]
        res = results[c]["out"].reshape(D, S1 + S2)
        # each (token, expert) output = sum of the two DFF-half partials
        # (cores 2k and 2k+1), scaled by the renormalized router weight
        na, nb_ = len(idx_lists[a]), len(idx_lists[b])
        out[idx_lists[a]] += res[:, :na].T * w_lists[a][:, None]
        out[idx_lists[b]] += res[:, S1 : S1 + nb_].T * w_lists[b][:, None]
    return out.reshape(B, L, D)


def _run_profiled(nc, in_maps, core_ids):
    """run_bass_kernel_spmd with trace=True, providing the NTFF hook that the
    agent image's antenv stub lacks, and skipping the artifact upload."""
    import sys
    import tempfile
    import types

    import concourse.bass_utils as bu

    if "antenv.axon_hooks" not in sys.modules:
        from trn_agent_boot.trn_boot import _ntff_profile_via_ctypes

        hook = _ntff_profile_via_ctypes("/opt/axon/libaxon_pjrt.so")
        mod = types.ModuleType("antenv.axon_hooks")
        mod.get_axon_ntff_profile_hook = lambda: hook
        mod.set_axon_ntff_profile_hook = lambda h: None
        sys.modules["antenv.axon_hooks"] = mod

    orig_upload = bu.upload_artifacts
    bu.upload_artifacts = lambda tmpdir: ""
    try:
        return run_bass_kernel_spmd(
            nc,
            in_maps,
            core_ids,
            trace=True,
            trace_cores=TRACE_CORES,
            tmpdir=tempfile.mkdtemp(prefix="moe_ntff_"),
        )
    finally:
        bu.upload_artifacts = orig_upload


if __name__ == "__main__":
    # smoke test with random data (no reference comparison)
    rng = np.random.default_rng(0)
    ins = {
        "x": rng.standard_normal((B, L, D), dtype=np.float32),
        "Wr": (rng.standard_normal((E, D)) * 0.02).astype(np.float32),
        "Wg": (rng.standard_normal((E, DFF, D)) * 0.02).astype(np.float32),
        "Wu": (rng.standard_normal((E, DFF, D)) * 0.02).astype(np.float32),
        "Wd": (rng.standard_normal((E, D, DFF)) * 0.02).astype(np.float32),
    }
    out = kernel(**ins)
    print("out", out.shape, out.dtype, float(np.abs(out).max()))


# revision 35
# speedup vs baseline: 1.0098x; 1.0098x over previous
"""MoE (top-2 routed GluMLP) Trainium2 kernel, expert+DFF-parallel over 8 cores.

Contract: kernel(**inputs) takes the FULL unsharded inputs
  x  [2, 2048, 1024] f32
  Wr [8, 1024] f32           router
  Wg [8, 4096, 1024] f32     gate proj per expert
  Wu [8, 4096, 1024] f32     up proj per expert
  Wd [8, 1024, 4096] f32     down proj per expert
and returns the FULL output [2, 2048, 1024] f32.

Strategy:
  - Routing (softmax + top-2 + renormalize) on host with jax on CPU using the
    exact ops of the reference, so expert selection and combine weights match
    the reference bit-for-bit.
  - Load balancing exploits that the GluMLP is additive along DFF: every
    expert is split into two 16-row-of-128 (DFF/2) halves. The 4 heaviest
    experts' halves fill segment 1 on the 8 cores (capacity S1 = max load);
    the 4 lightest fill segment 2 (capacity S2 = 5th-largest load). Per-core
    work is 16*(S1+S2) fm-tokens instead of 32*maxload — ~3% less.
  - Each (core, segment) runs a weighted GluMLP half over its tokens:
        h = relu(x @ WgT_half) * (x @ WuT_half);  out_part = Wd_half @ h
    with fp16 operands (same 10-bit mantissa as TF32) and fp32 PSUM
    accumulation. Phase C keeps TOKENS as the moving operand (no 128-token
    tile quantization) with the Wd halves fully resident in SBUF.
  - The renormalized router weight is applied on the HOST during the
    scatter-add (it commutes through the down-projection), which also sums
    the two DFF-half partials per (token, expert).

Env: MOE_MM_DT selects matmul operand dtype (f16 default, f32r, f32).
"""

import math
import os
from contextlib import ExitStack

import numpy as np

import concourse.bass as bass
import concourse.tile as tile
from concourse import bacc, mybir
from concourse.bass_utils import run_bass_kernel_spmd

B, L, D, E, TOPK, DFF = 2, 2048, 1024, 8, 2, 4096
T = B * L
NCORES = 8
P = 128
NB = 512          # matmul moving-operand block (one PSUM bank of fp32)
DC = D // P       # 8 contraction chunks over D (also 8 output d-tiles)
FM = DFF // P     # 32 f-tiles over DFF per expert
FMH = FM // 2     # 16 f-tiles per expert half
NSEG = 2

F32 = mybir.dt.float32
F32R = mybir.dt.float32r
F16 = mybir.dt.float16

# Set to True (e.g. from test.py) to run with NTFF tracing and print HW time.
PROFILE = False
TRACE_CORES = None  # e.g. list(range(8)) to profile every core
LAST_EXEC_NS = None
MM_DT = {"f32": F32, "f32r": F32R, "f16": F16}[os.environ.get("MOE_MM_DT", "f16")]


def _nblocks(tch):
    """Moving-dim blocks <=512. Every block must stream >= ~240 cols so the
    97ns (233-cycle) LDWEIGHTS stays hidden behind the col+6-cycle stream.
    The LAST block is made the smallest allowed: the final (block, d-tile)
    accumulation chain + drain of phase C is a pure serial tail."""
    k = max(1, math.ceil(tch / NB))
    if k == 1:
        return [(0, tch)]
    last = max(240, tch - NB * (k - 1))
    rest = tch - last
    base, rem = divmod(rest, k - 1)
    out, t = [], 0
    for i in range(k - 1):
        nb = base + (1 if i < rem else 0)
        out.append((t, nb))
        t += nb
    out.append((t, last))
    return out


def _build_nc(S1: int, S2: int):
    """Build the single-core Bass program (SPMD: all cores run the same NEFF).

    Segment s processes S_s tokens against one 16-f-tile expert half whose
    weights are input slice s."""
    nc = bacc.Bacc(
        "TRN2",
        target_bir_lowering=False,
        debug=False,
        enable_asserts=False,
        num_devices=NCORES,
    )
    mdt = MM_DT
    S = S1 + S2
    x_d = nc.dram_tensor("x", [P, DC, S], mdt, kind="ExternalInput").ap()
    wg_d = nc.dram_tensor("wg", [NSEG, FMH, P, DC, P], mdt, kind="ExternalInput").ap()
    wu_d = nc.dram_tensor("wu", [NSEG, FMH, P, DC, P], mdt, kind="ExternalInput").ap()
    wd_d = nc.dram_tensor("wd", [NSEG, FMH, P, D], mdt, kind="ExternalInput").ap()
    # out is [d-tile, d-inner, t] (transposed); host transposes, applies the
    # router weight, and sums the two DFF-half partials in the scatter-add.
    out_d = nc.dram_tensor("out", [DC, P, S], F32, kind="ExternalOutput").ap()

    with tile.TileContext(nc) as tc:
        with ExitStack() as ctx:
            _moe_body(ctx, tc, x_d, wg_d, wu_d, wd_d, out_d, S1, S2)
    nc.compile()
    return nc


def _moe_body(ctx, tc, x_d, wg_d, wu_d, wd_d, out_d, S1, S2):
    nc = tc.nc
    mdt = MM_DT
    S = S1 + S2
    segs = [(0, 0, S1), (1, S1, S2)]  # (seg idx, token offset, token count)

    xpool = ctx.enter_context(tc.tile_pool(name="xpool", bufs=1))
    hpool = ctx.enter_context(tc.tile_pool(name="hpool", bufs=1))
    # bufs=4: the in-loop wg/wu load for fm+4 can dispatch as soon as fm's
    # buffer frees — shallower prefetch slips behind the startup queue backlog
    wgupool = ctx.enter_context(tc.tile_pool(name="wgupool", bufs=4))
    # all 16 Wd tiles (2 segs x 8 d-tiles, 4KiB/partition each) stay resident:
    # Wd streams exactly once, behind phase B, and phase C never waits on DMA
    wdpool = ctx.enter_context(tc.tile_pool(name="wdpool", bufs=NSEG * DC))
    gpool = ctx.enter_context(tc.tile_pool(name="gpool", bufs=3))
    opool = ctx.enter_context(tc.tile_pool(name="opool", bufs=6))
    # One shared PSUM pool: phase B (ps_g/ps_u) and phase C (d-tiles) don't
    # overlap in time, so both get all 8 banks.
    psP = ctx.enter_context(tc.tile_pool(name="psP", bufs=8, space="PSUM"))

    # ---- startup DMA schedule -------------------------------------------
    # Bandwidth bound: ~3.2MB (segment-1 x + first two wg/wu pairs) must land
    # in the first ~13us. Full-tile transfers only — partition-split halves
    # and short rows both measurably reduce per-queue throughput. wg0 and
    # x dc=0 head different queues; wu0/wu1 ride later/slower slots.
    wg_p0 = wgupool.tile([P, DC, P], mdt, tag="wg", name="wg_pre0")
    nc.sync.dma_start(out=wg_p0, in_=wg_d[0, 0])
    x_sb = xpool.tile([P, DC, S], mdt, tag="x")
    nc.scalar.dma_start(out=x_sb[:, 0, :S1], in_=x_d[:, 0, :S1])
    wu_p0 = wgupool.tile([P, DC, P], mdt, tag="wu", name="wu_pre0")
    nc.scalar.dma_start(out=wu_p0, in_=wu_d[0, 0])
    for dc in range(1, DC):
        eng = [None, nc.gpsimd, nc.sync, nc.scalar, nc.gpsimd, nc.sync,
               nc.scalar, nc.sync][dc]
        eng.dma_start(out=x_sb[:, dc, :S1], in_=x_d[:, dc, :S1])
    wg_p1 = wgupool.tile([P, DC, P], mdt, tag="wg", name="wg_pre1")
    nc.sync.dma_start(out=wg_p1, in_=wg_d[0, 1])
    wu_p1 = wgupool.tile([P, DC, P], mdt, tag="wu", name="wu_pre1")
    nc.gpsimd.dma_start(out=wu_p1, in_=wu_d[0, 1])
    pre = [(wg_p0, wu_p0), (wg_p1, wu_p1)]

    h_all = hpool.tile([P, FMH, S], mdt, tag="h")
    wd_sbs = [[None] * DC for _ in range(NSEG)]

    # ---- Phase B: h[f, t] = relu(g) * u, f-major, per segment ------------
    for seg, off, sl in segs:
        for fm in range(FMH):
            if seg == 0 and fm < len(pre):
                wg_sb, wu_sb = pre[fm]
            else:
                wg_sb = wgupool.tile([P, DC, P], mdt, tag="wg")
                nc.sync.dma_start(out=wg_sb, in_=wg_d[seg, fm])
                wu_sb = wgupool.tile([P, DC, P], mdt, tag="wu")
                nc.scalar.dma_start(out=wu_sb, in_=wu_d[seg, fm])
            if seg == 0 and 3 <= fm < 3 + DC:
                # segment-2 x streams one stripe per fm iteration (needed
                # ~115us in; bursting them would starve the wg/wu prefetch)
                dc = fm - 3
                xeng = [nc.gpsimd, nc.sync, nc.scalar][dc % 3]
                xeng.dma_start(out=x_sb[:, dc, S1:], in_=x_d[:, dc, S1:])
            if seg == 1:
                # trickle the 16 resident Wd tiles behind segment 2's weight
                # stream; phase C runs segment 2 first, so its tiles load in
                # the first 8 slots
                ws, dt = divmod(fm, DC)
                ws = 1 - ws  # fm 0..7 -> seg2's wd, fm 8..15 -> seg1's
                wd_t = wdpool.tile([P, FMH, P], mdt, tag="wd", name=f"wd{ws}_{dt}")
                eng = [nc.sync, nc.scalar, nc.gpsimd][dt % 3]
                eng.dma_start(
                    out=wd_t,
                    in_=wd_d[ws, :, :, dt * P : dt * P + P].rearrange(
                        "f p d -> p f d"
                    ),
                )
                wd_sbs[ws][dt] = wd_t
            for nb0, nbl in _nblocks(sl):
                ts = slice(off + nb0, off + nb0 + nbl)
                ps_g = psP.tile([P, NB], F32, tag="ps")
                ps_u = psP.tile([P, NB], F32, tag="ps")
                for dc in range(DC):
                    nc.tensor.matmul(
                        ps_g[:, :nbl],
                        lhsT=wg_sb[:, dc, :],
                        rhs=x_sb[:, dc, ts],
                        start=(dc == 0),
                        stop=(dc == DC - 1),
                    )
                for dc in range(DC):
                    nc.tensor.matmul(
                        ps_u[:, :nbl],
                        lhsT=wu_sb[:, dc, :],
                        rhs=x_sb[:, dc, ts],
                        start=(dc == 0),
                        stop=(dc == DC - 1),
                    )
                g_sb = gpool.tile([P, NB], F32, tag="g")
                # relu on DVE (tensor_scalar_max), NOT the ACT engine: any
                # ACTIVATE instruction makes the scalar engine run a 1.28us
                # ACT_TABLE_LOAD in the preamble before its first DMA
                # dispatch, delaying the critical startup x stripes
                nc.vector.tensor_scalar_max(g_sb[:, :nbl], ps_g[:, :nbl], 0.0)
                nc.vector.tensor_mul(h_all[:, fm, ts], g_sb[:, :nbl], ps_u[:, :nbl])

    # ---- Phase C: out[d, t] = (Wd_half h)[d, t], tokens moving -----------
    # Segment 2 first: segment 1 ends with the small last block, so the
    # final serial chain + drain is as short as possible.
    st = 0
    for seg, off, sl in reversed(segs):
        blocks = _nblocks(sl)
        for bi, (nb0, nbl) in enumerate(blocks):
            # gpsimd's end-of-program SWDGE ring drain (~3.7us) starts only
            # after its last instruction: keep it off the final block's stores
            final_blk = seg == 0 and bi == len(blocks) - 1
            ts = slice(off + nb0, off + nb0 + nbl)
            for dt in range(DC):
                ps_o = psP.tile([P, NB], F32, tag="ps", name=f"ps_o{dt}")
                for fc in range(FMH):
                    nc.tensor.matmul(
                        ps_o[:, :nbl],
                        lhsT=wd_sbs[seg][dt][:, fc, :],
                        rhs=h_all[:, fc, ts],
                        start=(fc == 0),
                        stop=(fc == FMH - 1),
                    )
                o_sb = opool.tile([P, NB], F32, tag="o")
                # drains on DVE only (scalar.copy is ACTIVATE — see relu note;
                # DVE is otherwise idle in phase C). The final block drains in
                # partition halves so its first store (the serial program
                # tail) dispatches half a copy earlier.
                if final_blk:
                    hp = P // 2
                    nc.vector.tensor_scalar_mul(
                        o_sb[:hp, :nbl], ps_o[:hp, :nbl], 1.0
                    )
                    nc.vector.tensor_scalar_mul(
                        o_sb[hp:, :nbl], ps_o[hp:, :nbl], 1.0
                    )
                else:
                    nc.vector.tensor_scalar_mul(o_sb[:, :nbl], ps_o[:, :nbl], 1.0)
                # stores split into partition halves cycling the DMA queues
                # so the final block's drain isn't serialized
                engs = [nc.sync, nc.scalar] if final_blk else [
                    nc.gpsimd, nc.sync, nc.scalar]
                for hh in range(2):
                    rs = slice(hh * (P // 2), (hh + 1) * (P // 2))
                    engs[st % len(engs)].dma_start(
                        out=out_d[dt, rs, off + nb0 : off + nb0 + nbl],
                        in_=o_sb[rs, :nbl],
                    )
                    st += 1


_NC_CACHE: dict = {}


def _get_nc(S1: int, S2: int):
    if (S1, S2) not in _NC_CACHE:
        _NC_CACHE[(S1, S2)] = _build_nc(S1, S2)
    return _NC_CACHE[(S1, S2)]


def _round_tf32(a):
    """Round-to-nearest-even fp32 -> TF32 (10-bit mantissa), as np.float32."""
    u = a.astype(np.float32).view(np.uint32).astype(np.uint64)
    lsb = (u >> 13) & 1
    r = (u + 0x0FFF + lsb) & 0xFFFFE000
    return r.astype(np.uint32).view(np.float32)


def _mm_round(a):
    """Convert a host array to the dtype/value the device matmuls consume."""
    if MM_DT is F32R:
        return _round_tf32(a)
    if MM_DT is F16:
        return np.ascontiguousarray(a, dtype=np.float16)
    return np.ascontiguousarray(a, dtype=np.float32)


def _route_host(x, Wr):
    """Reference-identical routing on host (jax on CPU, same ops as reference).

    Returns (k_ids [T, K] int, k_w [T, K] f32).
    """
    import jax
    import jax.numpy as jnp

    cpu = jax.devices("cpu")[0]
    with jax.default_device(cpu):
        xt = jnp.asarray(x.reshape(T, D))
        logits = jnp.einsum("td,ed->te", xt, jnp.asarray(Wr))
        scores = jax.nn.softmax(logits, axis=-1)
        k_scores, k_ids = jax.lax.top_k(scores, TOPK)
        eps = jnp.finfo(x.dtype).eps
        k_w = k_scores / (k_scores.sum(axis=-1, keepdims=True) + eps)
        return np.asarray(k_ids), np.asarray(k_w)


def _prep_weights(Wg, Wu, Wd):
    """Per-expert weight tensors in device layouts (rounded to MM_DT)."""
    wg_r, wu_r, wd_r = [], [], []
    for e in range(len(Wg)):
        # Wg[e]: [DFF, D]; device wants [fm, p(d_inner), dc, f_inner]
        wgt = Wg[e].T.reshape(DC, P, FM, P).transpose(2, 1, 0, 3)
        wut = Wu[e].T.reshape(DC, P, FM, P).transpose(2, 1, 0, 3)
        # Wd[e]: [D, DFF]; device wants WdT = [fc, p(f_inner), d]
        wdt = Wd[e].T.reshape(FM, P, D)
        wg_r.append(_mm_round(np.ascontiguousarray(wgt, dtype=np.float32)))
        wu_r.append(_mm_round(np.ascontiguousarray(wut, dtype=np.float32)))
        wd_r.append(_mm_round(np.ascontiguousarray(wdt, dtype=np.float32)))
    return wg_r, wu_r, wd_r


def _xdev(xt_rows, cap):
    """Host [n, D] f32 -> device x layout [p(d_inner), dc, cap] in MM_DT."""
    xg = np.zeros((cap, D), dtype=np.float32)
    xg[: len(xt_rows)] = xt_rows
    xg_r = np.ascontiguousarray(
        xg.T.reshape(DC, P, cap).transpose(1, 0, 2), dtype=np.float32
    )
    return _mm_round(xg_r)


def kernel(x, Wr, Wg, Wu, Wd):
    global LAST_EXEC_NS
    x = np.asarray(x, dtype=np.float32)
    Wr = np.asarray(Wr, dtype=np.float32)
    Wg = np.asarray(Wg, dtype=np.float32)
    Wu = np.asarray(Wu, dtype=np.float32)
    Wd = np.asarray(Wd, dtype=np.float32)

    k_ids, k_w = _route_host(x, Wr)
    xt = x.reshape(T, D)

    # Gather per-expert token lists (each token appears once per selected expert).
    idx_lists, w_lists = [], []
    for e in range(E):
        tmask = k_ids == e                       # [T, K]
        tok = np.nonzero(tmask.any(axis=1))[0]   # unique tokens routed to e
        wvals = (k_w * tmask).sum(axis=1)[tok].astype(np.float32)
        idx_lists.append(tok)
        w_lists.append(wvals)

    loads = np.array([len(t) for t in idx_lists])
    order = np.argsort(-loads, kind="stable")
    # DFF-split balance: segment 1 carries the 4 heaviest experts (two cores
    # per expert, one 16-f-tile half each), segment 2 the 4 lightest.
    S1 = max(P, (int(loads[order[0]]) + 7) // 8 * 8)
    S2 = max(P, (int(loads[order[4]]) + 7) // 8 * 8)
    assert S1 + S2 <= 2304, f"capacity {S1}+{S2} exceeds SBUF budget"

    wg_r, wu_r, wd_r = _prep_weights(Wg, Wu, Wd)

    in_maps = []
    core_assign = []  # (expert_a, expert_b) per core
    for c in range(NCORES):
        a, ha = int(order[c // 2]), c % 2
        b, hb = int(order[4 + c // 2]), c % 2
        xa = _xdev(xt[idx_lists[a]], S1)
        xb = _xdev(xt[idx_lists[b]], S2)
        in_maps.append(
            {
                "x": np.concatenate([xa, xb], axis=2),
                "wg": np.ascontiguousarray(
                    np.stack([wg_r[a][FMH * ha : FMH * (ha + 1)],
                              wg_r[b][FMH * hb : FMH * (hb + 1)]])
                ),
                "wu": np.ascontiguousarray(
                    np.stack([wu_r[a][FMH * ha : FMH * (ha + 1)],
                              wu_r[b][FMH * hb : FMH * (hb + 1)]])
                ),
                "wd": np.ascontiguousarray(
                    np.stack([wd_r[a][FMH * ha : FMH * (ha + 1)],
                              wd_r[b][FMH * hb : FMH * (hb + 1)]])
                ),
            }
        )
        core_assign.append((a, b))

    nc = _get_nc(S1, S2)
    core_ids = list(range(NCORES))
    if PROFILE:
        res = _run_profiled(nc, in_maps, core_ids)
        LAST_EXEC_NS = res.exec_time_ns
        results = res.results
    else:
        results = run_bass_kernel_spmd(nc, in_maps, core_ids).results

    out = np.zeros((T, D), dtype=np.float32)
    for c in range(NCORES):
        a, b = core_assign[c]
        res = results[c]["out"].reshape(D, S1 + S2)
        # each (token, expert) output = sum of the two DFF-half partials
        # (cores 2k and 2k+1), scaled by the renormalized router weight
        na, nb_ = len(idx_lists[a]), len(idx_lists[b])
        out[idx_lists[a]] += res[:, :na].T * w_lists[a][:, None]
        out[idx_lists[b]] += res[:, S1 : S1 + nb_].T * w_lists[b][:, None]
    return out.reshape(B, L, D)


def _run_profiled(nc, in_maps, core_ids):
    """run_bass_kernel_spmd with trace=True, providing the NTFF hook that the
    agent image's antenv stub lacks, and skipping the artifact upload."""
    import sys
    import tempfile
    import types

    import concourse.bass_utils as bu

    if "antenv.axon_hooks" not in sys.modules:
        from trn_agent_boot.trn_boot import _ntff_profile_via_ctypes

        hook = _ntff_profile_via_ctypes("/opt/axon/libaxon_pjrt.so")
        mod = types.ModuleType("antenv.axon_hooks")
        mod.get_axon_ntff_profile_hook = lambda: hook
        mod.set_axon_ntff_profile_hook = lambda h: None
        sys.modules["antenv.axon_hooks"] = mod

    orig_upload = bu.upload_artifacts
    bu.upload_artifacts = lambda tmpdir: ""
    try:
        return run_bass_kernel_spmd(
            nc,
            in_maps,
            core_ids,
            trace=True,
            trace_cores=TRACE_CORES,
            tmpdir=tempfile.mkdtemp(prefix="moe_ntff_"),
        )
    finally:
        bu.upload_artifacts = orig_upload


if __name__ == "__main__":
    # smoke test with random data (no reference comparison)
    rng = np.random.default_rng(0)
    ins = {
        "x": rng.standard_normal((B, L, D), dtype=np.float32),
        "Wr": (rng.standard_normal((E, D)) * 0.02).astype(np.float32),
        "Wg": (rng.standard_normal((E, DFF, D)) * 0.02).astype(np.float32),
        "Wu": (rng.standard_normal((E, DFF, D)) * 0.02).astype(np.float32),
        "Wd": (rng.standard_normal((E, D, DFF)) * 0.02).astype(np.float32),
    }
    out = kernel(**ins)
    print("out", out.shape, out.dtype, float(np.abs(out).max()))
